# revision 1
# baseline (speedup 1.0000x reference)
"""CoAttention kernel for 8 Trainium2 NeuronCores.

Sharding: data-parallel over batch B=8 -> one batch per core. BatchNorm
batch-stats are computed per-core and summed with a mid-kernel AllReduce
(sum / sum-of-squares per channel, 2 x [128,12] f32 per branch).

Per-core dataflow (batch b, L=2048, D=768):
  q^T, kv^T in [D, L] layout via SWDGE f32->bf16 cast load + HWDGE xbar
  transpose.  For each branch: S^T[m,l] = kv^T.T @ q^T tiles (PE, bf16),
  exp on ACT (fused * 1/sqrt(D)) -> E^T bf16; softmax denominators via
  ones-matmuls; ctx^T = kv_nat.T @ E^T accumulated in PSUM, normalized by
  1/s broadcast on the ctx PSUM->SBUF copy; y^T = W^T.T @ ctx^T; y -> DRAM
  bounce (bf16) + bn_stats.  AllReduce of per-channel stats, then
  BN+PReLU fused into two ACT Relu passes (prelu(z)=relu(z)-a*relu(-z)),
  accumulated into acc^T = res_a^T + res_v^T.  Tail: PE-transpose acc to
  natural layout, add fp32 x re-read, LayerNorm over D, DMA out.
"""
import os
import sys

for _p in ("/opt/trn_rl_repo",):
    if _p not in sys.path and os.path.isdir(_p):
        sys.path.append(_p)

import numpy as np

import concourse.bass as bass
import concourse.mybir as mybir
import concourse.tile as tile
from concourse import bacc
from concourse.bass_utils import run_bass_kernel_spmd
from concourse.masks import make_identity

L, B, D = 2048, 8, 768
N_CORES = 8
LT = L // 128          # 16 l-tiles (128 queries)
DT = D // 128          # 6 d-tiles
MT = L // 128          # 16 m-tiles (keys)
LBS = 512              # l-block size
NLB = L // LBS         # 4 l-blocks
EPS_BN = 1e-5
EPS_LN = 1e-5
SCALE = 1.0 / float(np.sqrt(D))
F32 = mybir.dt.float32
BF16 = mybir.dt.bfloat16
AF = mybir.ActivationFunctionType
ALU = mybir.AluOpType

_CACHED_NC = None


def _build_nc():
    nc = bacc.Bacc("TRN2", target_bir_lowering=False, debug=False,
                   num_devices=N_CORES)

    # Per-core DRAM I/O (core i gets batch i slices of x/x_a/x_v).
    xq_d = nc.dram_tensor("xq", [L, D], F32, kind="ExternalInput")
    xa_d = nc.dram_tensor("xa", [L, D], F32, kind="ExternalInput")
    xv_d = nc.dram_tensor("xv", [L, D], F32, kind="ExternalInput")
    Wa_d = nc.dram_tensor("Wa", [D, D], F32, kind="ExternalInput")
    Wv_d = nc.dram_tensor("Wv", [D, D], F32, kind="ExternalInput")
    ba_d = nc.dram_tensor("ba", [D], F32, kind="ExternalInput")
    bv_d = nc.dram_tensor("bv", [D], F32, kind="ExternalInput")
    bnag_d = nc.dram_tensor("bnag", [D], F32, kind="ExternalInput")
    bnab_d = nc.dram_tensor("bnab", [D], F32, kind="ExternalInput")
    bnvg_d = nc.dram_tensor("bnvg", [D], F32, kind="ExternalInput")
    bnvb_d = nc.dram_tensor("bnvb", [D], F32, kind="ExternalInput")
    pa_d = nc.dram_tensor("pa", [1], F32, kind="ExternalInput")
    pv_d = nc.dram_tensor("pv", [1], F32, kind="ExternalInput")
    lng_d = nc.dram_tensor("lng", [D], F32, kind="ExternalInput")
    lnb_d = nc.dram_tensor("lnb", [D], F32, kind="ExternalInput")
    out_d = nc.dram_tensor("out", [L, D], F32, kind="ExternalOutput")

    def bcast_ap(t, n):
        a = t.ap() if hasattr(t, "ap") and callable(getattr(t, "ap")) else t
        return bass.AP(tensor=a.tensor, offset=a.offset,
                       ap=[[0, 128]] + [list(x) for x in a.ap])

    from contextlib import ExitStack
    with ExitStack() as ctx:
        tc = ctx.enter_context(tile.TileContext(nc))
        constp = ctx.enter_context(tc.tile_pool(name="const", bufs=1))
        natp = ctx.enter_context(tc.tile_pool(name="nat", bufs=24))      # [128,768] bf16
        qtp = ctx.enter_context(tc.tile_pool(name="qt", bufs=4))         # [128,6,512] bf16
        kvtp = ctx.enter_context(tc.tile_pool(name="kvt", bufs=32))      # [128,6,128] bf16
        wtp = ctx.enter_context(tc.tile_pool(name="wt", bufs=1))
        wnatp = ctx.enter_context(tc.tile_pool(name="wnat", bufs=6))         # [128,6,768] bf16
        ep = ctx.enter_context(tc.tile_pool(name="e", bufs=1))           # [128,16,512] bf16
        ctxp = ctx.enter_context(tc.tile_pool(name="ctx", bufs=6))       # [128,512] bf16
        rbp = ctx.enter_context(tc.tile_pool(name="rb", bufs=1))         # [128,512] f32
        ysp = ctx.enter_context(tc.tile_pool(name="ystage", bufs=3))     # [128,512] bf16
        ybkp = ctx.enter_context(tc.tile_pool(name="ybk", bufs=4))       # [128,2048] bf16
        rtp = ctx.enter_context(tc.tile_pool(name="rtmp", bufs=4))       # [128,2048] bf16
        accp = ctx.enter_context(tc.tile_pool(name="accs", bufs=6))      # [128,2048] bf16
        statp = ctx.enter_context(tc.tile_pool(name="stats", bufs=1))
        smallp = ctx.enter_context(tc.tile_pool(name="small", bufs=1))
        finp = ctx.enter_context(tc.tile_pool(name="fin", bufs=2))       # [128,768] f32
        lnsp = ctx.enter_context(tc.tile_pool(name="lns", bufs=3))
        dramp = ctx.enter_context(tc.tile_pool(name="dram", bufs=1, space="DRAM"))
        ps_s = ctx.enter_context(tc.tile_pool(name="ps_s", bufs=2, space="PSUM"))
        ps_sum = ctx.enter_context(tc.tile_pool(name="ps_sum", bufs=1, space="PSUM"))
        ps_acc = ctx.enter_context(tc.tile_pool(name="ps_acc", bufs=4, space="PSUM"))
        if True:
            # ---------- constants / params ----------
            ident = constp.tile([128, 128], BF16)
            make_identity(nc, ident)
            ones = constp.tile([128, 1], BF16)
            nc.vector.memset(ones[:], 1.0)
            epsbn = constp.tile([128, 1], F32)
            nc.vector.memset(epsbn[:], EPS_BN)
            epsln = constp.tile([128, 1], F32)
            nc.vector.memset(epsln[:], EPS_LN)

            def load_pt(t):  # [D] -> [128, DT] with col et, row p = t[et*128+p]
                s = constp.tile([128, DT], F32)
                nc.scalar.dma_start(out=s[:], in_=t.ap().rearrange("(t p) -> p t", p=128))
                return s

            ba_s, bv_s = load_pt(ba_d), load_pt(bv_d)
            bnag_s, bnab_s = load_pt(bnag_d), load_pt(bnab_d)
            bnvg_s, bnvb_s = load_pt(bnvg_d), load_pt(bnvb_d)
            pa_s = constp.tile([128, 1], F32)
            nc.gpsimd.dma_start(out=pa_s[:], in_=bcast_ap(pa_d, 1))
            pv_s = constp.tile([128, 1], F32)
            nc.gpsimd.dma_start(out=pv_s[:], in_=bcast_ap(pv_d, 1))
            lng_s = constp.tile([128, D], F32)
            nc.gpsimd.dma_start(out=lng_s[:], in_=bcast_ap(lng_d, D))
            lnb_s = constp.tile([128, D], F32)
            nc.gpsimd.dma_start(out=lnb_s[:], in_=bcast_ap(lnb_d, D))

            # DRAM bounce buffers
            yD0 = dramp.tile([128, DT, L], BF16, tag="yD0")
            yD1 = dramp.tile([128, DT, L], BF16, tag="yD1")
            arin0 = dramp.tile([128, 2 * DT], F32, tag="arin0")
            arin1 = dramp.tile([128, 2 * DT], F32, tag="arin1")
            arout0 = dramp.tile([128, 2 * DT], F32, tag="arout0")
            arout1 = dramp.tile([128, 2 * DT], F32, tag="arout1")
            yD = {0: yD0, 1: yD1}
            arin = {0: arin0, 1: arin1}
            arout = {0: arout0, 1: arout1}

            # ---------- load + transpose inputs ----------
            # gpsimd (SWDGE cast) queue order: q0-3, kva0-15, q4-15, natv0-15
            # sync (xbar) ring carries ONLY transposes: q0-3, kva, q4-15, Wa, kvv, Wv
            def cast_load(src_d, c, name):
                natc = natp.tile([128, D], BF16, tag="nat", name=name)
                nc.gpsimd.dma_start(
                    out=natc[:], in_=src_d.ap()[c * 128:(c + 1) * 128, :])
                return natc

            qT = [qtp.tile([128, DT, LBS], BF16, tag="qT", name=f"qT{i}") for i in range(NLB)]
            kvT_a = [kvtp.tile([128, DT, 128], BF16, tag="kvT", name=f"kvTa{c}") for c in range(MT)]
            kvT_v = [kvtp.tile([128, DT, 128], BF16, tag="kvT", name=f"kvTv{c}") for c in range(MT)]
            nat_q = [None] * LT
            nat_a = [None] * LT
            for c in range(4):
                nat_q[c] = cast_load(xq_d, c, f"natq{c}")
            for c in range(MT):
                nat_a[c] = cast_load(xa_d, c, f"nata{c}")
            for c in range(4, LT):
                nat_q[c] = cast_load(xq_d, c, f"natq{c}")
            nat_v = [cast_load(xv_d, c, f"natv{c}") for c in range(MT)]

            def tq(c):
                nc.sync.dma_start_transpose(
                    qT[c // 4][:, :, (c % 4) * 128:(c % 4 + 1) * 128], nat_q[c][:])

            for c in range(4):
                tq(c)
            for c in range(8):
                nc.sync.dma_start_transpose(kvT_a[c][:, :, :], nat_a[c][:])
            for c in range(4, 8):
                tq(c)
            for c in range(8, MT):
                nc.sync.dma_start_transpose(kvT_a[c][:, :, :], nat_a[c][:])
            for c in range(8, LT):
                tq(c)

            def branch(bi, W_d, b_s, bng_s, bnb_s, alpha_s, acc_tiles, kvT, nat_kv):
                # W^T
                wnat = []
                for c6 in range(DT):
                    wn = wnatp.tile([128, D], BF16, tag="wnat")
                    nc.gpsimd.dma_start(
                        out=wn[:], in_=W_d.ap()[c6 * 128:(c6 + 1) * 128, :])
                    wnat.append(wn)
                WT = wtp.tile([128, DT, D], BF16, tag="WT")
                for c6 in range(DT):
                    nc.sync.dma_start_transpose(
                        WT[:, :, c6 * 128:(c6 + 1) * 128], wnat[c6][:])

                statsr = statp.tile([128, DT, NLB, 6], F32, tag="statsr")
                for lb in range(NLB):
                    E = ep.tile([128, MT, LBS], BF16, tag="E")
                    for mt in range(MT):
                        S = ps_s.tile([128, LBS], F32, tag="S")
                        for dt in range(DT):
                            nc.tensor.matmul(
                                S[:], kvT[mt][:, dt, :], qT[lb][:, dt, :],
                                start=(dt == 0), stop=(dt == DT - 1))
                        nc.scalar.activation(out=E[:, mt, :], in_=S[:],
                                             func=AF.Exp, scale=SCALE)
                    s_ps = ps_sum.tile([1, LBS], F32, tag="ssum")
                    for mt in range(MT):
                        nc.tensor.matmul(s_ps[:], ones[:], E[:, mt, :],
                                         start=(mt == 0), stop=(mt == MT - 1))
                    rsb = smallp.tile([1, LBS], F32, tag="rsb")
                    nc.vector.reciprocal(rsb[:], s_ps[:])
                    rbd = dramp.tile([LBS], F32, tag="rbd")
                    nc.scalar.dma_start(out=rbd[:], in_=rsb[:])
                    rbc = rbp.tile([128, LBS], F32, tag="rbc")
                    nc.scalar.dma_start(out=rbc[:], in_=bcast_ap(rbd, LBS))

                    ctx_sb = []
                    for dt in range(DT):
                        cps = ps_acc.tile([128, LBS], F32, tag="acc")
                        for mt in range(MT):
                            nc.tensor.matmul(
                                cps[:], nat_kv[mt][:, dt * 128:(dt + 1) * 128],
                                E[:, mt, :], start=(mt == 0), stop=(mt == MT - 1))
                        csb = ctxp.tile([128, LBS], BF16, tag="ctx")
                        nc.vector.tensor_copy(csb[:], cps[:])
                        ctx_sb.append(csb)
                    for et in range(DT):
                        yps = ps_acc.tile([128, LBS], F32, tag="acc")
                        for dt in range(DT):
                            nc.tensor.matmul(
                                yps[:], WT[:, dt, et * 128:(et + 1) * 128],
                                ctx_sb[dt][:], start=(dt == 0), stop=(dt == DT - 1))
                        ysb = ysp.tile([128, LBS], BF16, tag="ys")
                        nc.vector.tensor_mul(ysb[:], yps[:], rbc[:])
                        nc.vector.tensor_scalar(
                            out=ysb[:], in0=ysb[:], scalar1=b_s[:, et:et + 1],
                            scalar2=None, op0=ALU.add)
                        nc.vector.bn_stats(out=statsr[:, et, lb, :], in_=ysb[:])
                        nc.gpsimd.dma_start(
                            out=yD[bi][:, et, lb * LBS:(lb + 1) * LBS], in_=ysb[:])

                # per-core stats -> sums -> AllReduce
                mv = smallp.tile([128, DT, 2], F32, tag=f"mv{bi}")
                for et in range(DT):
                    nc.vector.bn_aggr(out=mv[:, et, :], in_=statsr[:, et, :, :])
                arin_s = smallp.tile([128, 2 * DT], F32, tag=f"ari{bi}")
                nc.vector.tensor_scalar(
                    out=arin_s[:, 0:DT], in0=mv[:, :, 0], scalar1=float(L),
                    scalar2=None, op0=ALU.mult)
                tmp = smallp.tile([128, DT], F32, tag=f"tmp{bi}")
                nc.vector.tensor_mul(tmp[:], mv[:, :, 0], mv[:, :, 0])
                nc.vector.tensor_add(tmp[:], tmp[:], mv[:, :, 1])
                nc.vector.tensor_scalar(
                    out=arin_s[:, DT:2 * DT], in0=tmp[:], scalar1=float(L),
                    scalar2=None, op0=ALU.mult)
                nc.scalar.dma_start(out=arin[bi][:], in_=arin_s[:])
                nc.gpsimd.collective_compute(
                    "AllReduce", ALU.add,
                    replica_groups=[list(range(N_CORES))],
                    ins=[arin[bi].opt()], outs=[arout[bi].opt()])
                gs = smallp.tile([128, 2 * DT], F32, tag=f"gs{bi}")
                nc.scalar.dma_start(out=gs[:], in_=arout[bi][:])

                inv_n = 1.0 / float(L * N_CORES)
                gm = smallp.tile([128, DT], F32, tag=f"gm{bi}")
                nc.vector.tensor_scalar(out=gm[:], in0=gs[:, 0:DT],
                                        scalar1=inv_n, scalar2=None, op0=ALU.mult)
                gvar = smallp.tile([128, DT], F32, tag=f"gv{bi}")
                nc.vector.tensor_scalar(out=gvar[:], in0=gs[:, DT:2 * DT],
                                        scalar1=inv_n, scalar2=None, op0=ALU.mult)
                tmp2 = smallp.tile([128, DT], F32, tag=f"t2{bi}")
                nc.vector.tensor_mul(tmp2[:], gm[:], gm[:])
                nc.vector.tensor_sub(gvar[:], gvar[:], tmp2[:])
                std = smallp.tile([128, DT], F32, tag=f"sd{bi}")
                nc.scalar.activation(out=std[:], in_=gvar[:], func=AF.Sqrt,
                                     bias=epsbn[:], scale=1.0)
                rstd = smallp.tile([128, DT], F32, tag=f"rs{bi}")
                nc.vector.reciprocal(rstd[:], std[:])
                sc1 = smallp.tile([128, DT], F32, tag=f"s1{bi}")
                nc.vector.tensor_mul(sc1[:], bng_s[:], rstd[:])
                sh1 = smallp.tile([128, DT], F32, tag=f"h1{bi}")
                nc.vector.tensor_mul(sh1[:], gm[:], sc1[:])
                nc.vector.tensor_sub(sh1[:], bnb_s[:], sh1[:])
                sc2 = smallp.tile([128, DT], F32, tag=f"s2{bi}")
                nc.vector.tensor_scalar(out=sc2[:], in0=sc1[:], scalar1=alpha_s[:],
                                        scalar2=-1.0, op0=ALU.mult, op1=ALU.mult)
                sh2 = smallp.tile([128, DT], F32, tag=f"h2{bi}")
                nc.vector.tensor_scalar(out=sh2[:], in0=sh1[:], scalar1=alpha_s[:],
                                        scalar2=-1.0, op0=ALU.mult, op1=ALU.mult)

                # BN + PReLU applied from the DRAM bounce, accumulate res^T.
                # Chunked by l-block so the tail transposes can pipeline.
                if bi == 0:
                    for et in range(DT):
                        acc_tiles[et] = accp.tile([128, L], BF16, tag="acc_sb",
                                                  name=f"accsb{et}")
                for lc in range(NLB):
                    lsl = slice(lc * LBS, (lc + 1) * LBS)
                    for et in range(DT):
                        ybk = ybkp.tile([128, LBS], BF16, tag="ybk")
                        nc.gpsimd.dma_start(out=ybk[:], in_=yD[bi][:, et, lsl])
                        r1 = rtp.tile([128, LBS], BF16, tag="rt")
                        nc.scalar.activation(out=r1[:], in_=ybk[:], func=AF.Relu,
                                             scale=sc1[:, et:et + 1], bias=sh1[:, et:et + 1])
                        r2 = rtp.tile([128, LBS], BF16, tag="rt")
                        nc.scalar.activation(out=r2[:], in_=ybk[:], func=AF.Relu,
                                             scale=sc2[:, et:et + 1], bias=sh2[:, et:et + 1])
                        if bi == 0:
                            nc.vector.tensor_sub(acc_tiles[et][:, lsl], r1[:], r2[:])
                        else:
                            nc.vector.tensor_add(acc_tiles[et][:, lsl],
                                                 acc_tiles[et][:, lsl], r1[:])
                            nc.vector.tensor_sub(acc_tiles[et][:, lsl],
                                                 acc_tiles[et][:, lsl], r2[:])

            acc_tiles = [None] * DT
            branch(0, Wa_d, ba_s, bnag_s, bnab_s, pa_s, acc_tiles, kvT_a, nat_a)
            for c in range(MT):
                nc.sync.dma_start_transpose(kvT_v[c][:, :, :], nat_v[c][:])
            branch(1, Wv_d, bv_s, bnvg_s, bnvb_s, pv_s, acc_tiles, kvT_v, nat_v)

            # ---------- tail: transpose + residual + LayerNorm ----------
            for lt in range(LT):
                xr = finp.tile([128, D], F32, tag="xr")
                nc.gpsimd.dma_start(out=xr[:], in_=xq_d.ap()[lt * 128:(lt + 1) * 128, :])
                onat = finp.tile([128, D], F32, tag="onat")
                for dt in range(DT):
                    tp = ps_s.tile([128, 128], BF16, tag="S")
                    nc.tensor.transpose(tp[:], acc_tiles[dt][:, lt * 128:(lt + 1) * 128],
                                        ident[:])
                    nc.vector.tensor_add(onat[:, dt * 128:(dt + 1) * 128], tp[:],
                                         xr[:, dt * 128:(dt + 1) * 128])
                lns = lnsp.tile([128, 3, 6], F32, tag="lns")
                for g3 in range(3):
                    nc.vector.bn_stats(out=lns[:, g3, :],
                                       in_=onat[:, g3 * 256:(g3 + 1) * 256])
                mvl = lnsp.tile([128, 2], F32, tag="mvl")
                nc.vector.bn_aggr(out=mvl[:], in_=lns[:])
                stdl = lnsp.tile([128, 1], F32, tag="stdl")
                nc.scalar.activation(out=stdl[:], in_=mvl[:, 1:2], func=AF.Sqrt,
                                     bias=epsln[:], scale=1.0)
                rstdl = lnsp.tile([128, 1], F32, tag="rstdl")
                nc.vector.reciprocal(rstdl[:], stdl[:])
                nc.vector.tensor_scalar(out=onat[:], in0=onat[:],
                                        scalar1=mvl[:, 0:1], scalar2=rstdl[:],
                                        op0=ALU.subtract, op1=ALU.mult)
                nc.vector.tensor_mul(onat[:], onat[:], lng_s[:])
                nc.vector.tensor_add(onat[:], onat[:], lnb_s[:])
                nc.gpsimd.dma_start(out=out_d.ap()[lt * 128:(lt + 1) * 128, :], in_=onat[:])

    nc.compile()
    return nc


def _get_nc():
    global _CACHED_NC
    if _CACHED_NC is None:
        _CACHED_NC = _build_nc()
    return _CACHED_NC


def kernel(**inputs):
    nc = _get_nc()
    x_a = np.asarray(inputs["x_a"], np.float32)
    x_v = np.asarray(inputs["x_v"], np.float32)
    x = np.asarray(inputs["x"], np.float32)
    shared = {
        "Wa": np.ascontiguousarray(inputs["W_a"], np.float32),
        "Wv": np.ascontiguousarray(inputs["W_v"], np.float32),
        "ba": np.ascontiguousarray(inputs["b_a"], np.float32),
        "bv": np.ascontiguousarray(inputs["b_v"], np.float32),
        "bnag": np.ascontiguousarray(inputs["bn_a_g"], np.float32),
        "bnab": np.ascontiguousarray(inputs["bn_a_b"], np.float32),
        "bnvg": np.ascontiguousarray(inputs["bn_v_g"], np.float32),
        "bnvb": np.ascontiguousarray(inputs["bn_v_b"], np.float32),
        "pa": np.ascontiguousarray(inputs["prelu_a"], np.float32),
        "pv": np.ascontiguousarray(inputs["prelu_v"], np.float32),
        "lng": np.ascontiguousarray(inputs["ln_g"], np.float32),
        "lnb": np.ascontiguousarray(inputs["ln_b"], np.float32),
    }
    in_maps = []
    for b in range(N_CORES):
        m = dict(shared)
        m["xq"] = np.ascontiguousarray(x[:, b, :])
        m["xa"] = np.ascontiguousarray(x_a[:, b, :])
        m["xv"] = np.ascontiguousarray(x_v[:, b, :])
        in_maps.append(m)
    trace = bool(int(os.environ.get("COATT_TRACE", "0")))
    res = run_bass_kernel_spmd(nc, in_maps, core_ids=list(range(N_CORES)),
                               trace=trace)
    kernel.last_results = res
    out = np.stack([res.results[b]["out"] for b in range(N_CORES)], axis=1)
    return out.astype(np.float32)



# revision 6
# speedup vs baseline: 1.2495x; 1.2495x over previous
"""CoAttention kernel for 8 Trainium2 NeuronCores.

Sharding: data-parallel over batch B=8 -> one batch per core. BatchNorm
batch-stats are computed per-core and summed with a mid-kernel AllReduce
(sum / sum-of-squares per channel, 2 x [128,12] f32 per branch).

v2 schedule vs v1:
  - Input cast-loads batched 4 l-tiles per SWDGE descriptor (gpsimd ring);
    xbar transposes split across BOTH HWDGE rings (scalar: kvT_a; sync:
    qT, WT_a, kvT_v, WT_v, out writes) so the supply chain feeds the PE
    from ~8us instead of trickling for 300us.
  - Emission order: branch_a compute -> AR_a trigger -> branch_v compute
    -> AR_v trigger -> coefs_a -> apply_a -> coefs_v -> apply_v -> tail.
    AR_a hides under branch_v's matmuls, AR_v hides under apply_a, so
    only apply_v + LN remain exposed after the last matmul.
  - SBUF pools sized to fit: qT slots are recycled for the acc tiles
    (same tag), kvT_v reuses kvT_a slots, W^T single-buffered.
"""
import os
import sys

for _p in ("/opt/trn_rl_repo",):
    if _p not in sys.path and os.path.isdir(_p):
        sys.path.append(_p)

import numpy as np

import concourse.bass as bass
import concourse.mybir as mybir
import concourse.tile as tile
from concourse import bacc
from concourse.bass_utils import run_bass_kernel_spmd
from concourse.masks import make_identity

L, B, D = 2048, 8, 768
N_CORES = 8
LT = L // 128          # 16 l-tiles (128 queries)
DT = D // 128          # 6 d-tiles
MT = L // 128          # 16 m-tiles (keys)
LBS = 512              # l-block size
NLB = L // LBS         # 4 l-blocks
CH = 4                 # l-tiles per load chunk
NCH = LT // CH         # 4 chunks per input tensor
WCH = 3                # r-tiles per W load chunk
EPS_BN = 1e-5
EPS_LN = 1e-5
SCALE = 1.0 / float(np.sqrt(D))
F32 = mybir.dt.float32
BF16 = mybir.dt.bfloat16
AF = mybir.ActivationFunctionType
ALU = mybir.AluOpType

_CACHED_NC = None


def _build_nc():
    nc = bacc.Bacc("TRN2", target_bir_lowering=False, debug=False,
                   num_devices=N_CORES)

    xq_d = nc.dram_tensor("xq", [L, D], F32, kind="ExternalInput")
    xa_d = nc.dram_tensor("xa", [L, D], F32, kind="ExternalInput")
    xv_d = nc.dram_tensor("xv", [L, D], F32, kind="ExternalInput")
    Wa_d = nc.dram_tensor("Wa", [D, D], F32, kind="ExternalInput")
    Wv_d = nc.dram_tensor("Wv", [D, D], F32, kind="ExternalInput")
    ba_d = nc.dram_tensor("ba", [D], F32, kind="ExternalInput")
    bv_d = nc.dram_tensor("bv", [D], F32, kind="ExternalInput")
    bnag_d = nc.dram_tensor("bnag", [D], F32, kind="ExternalInput")
    bnab_d = nc.dram_tensor("bnab", [D], F32, kind="ExternalInput")
    bnvg_d = nc.dram_tensor("bnvg", [D], F32, kind="ExternalInput")
    bnvb_d = nc.dram_tensor("bnvb", [D], F32, kind="ExternalInput")
    pa_d = nc.dram_tensor("pa", [1], F32, kind="ExternalInput")
    pv_d = nc.dram_tensor("pv", [1], F32, kind="ExternalInput")
    lng_d = nc.dram_tensor("lng", [D], F32, kind="ExternalInput")
    lnb_d = nc.dram_tensor("lnb", [D], F32, kind="ExternalInput")
    out_d = nc.dram_tensor("out", [L, D], F32, kind="ExternalOutput")

    def bcast_ap(t, n):
        a = t.ap() if hasattr(t, "ap") and callable(getattr(t, "ap")) else t
        return bass.AP(tensor=a.tensor, offset=a.offset,
                       ap=[[0, 128]] + [list(x) for x in a.ap])

    from contextlib import ExitStack
    with ExitStack() as ctx:
        tc = ctx.enter_context(tile.TileContext(nc))
        constp = ctx.enter_context(tc.tile_pool(name="const", bufs=1))
        stgp = ctx.enter_context(tc.tile_pool(name="stg", bufs=2))      # natq/W chunks
        natap = ctx.enter_context(tc.tile_pool(name="nata", bufs=4))    # [128,4,768] bf16
        natvp = ctx.enter_context(tc.tile_pool(name="natv", bufs=4))
        qaccp = ctx.enter_context(tc.tile_pool(name="qacc", bufs=6))    # qT then acc
        kvtp = ctx.enter_context(tc.tile_pool(name="kvt", bufs=18))     # [128,6,128] bf16
        wtp = ctx.enter_context(tc.tile_pool(name="wt", bufs=1))        # [128,6,768] bf16
        ep = ctx.enter_context(tc.tile_pool(name="e", bufs=1))          # [128,16,512] bf16
        ctxp = ctx.enter_context(tc.tile_pool(name="ctx", bufs=6))      # [128,512] bf16
        rbp = ctx.enter_context(tc.tile_pool(name="rb", bufs=1))        # [128,512] f32
        ysp = ctx.enter_context(tc.tile_pool(name="ys", bufs=2))        # [128,6,512] bf16
        ybkp = ctx.enter_context(tc.tile_pool(name="ybk", bufs=1))      # [128,6,512] bf16
        rtp = ctx.enter_context(tc.tile_pool(name="rtmp", bufs=4))      # [128,512] bf16
        statp = ctx.enter_context(tc.tile_pool(name="stats", bufs=2))
        smallp = ctx.enter_context(tc.tile_pool(name="small", bufs=1))
        xrp = ctx.enter_context(tc.tile_pool(name="xr", bufs=1))        # [128,2,768] f32
        onatp = ctx.enter_context(tc.tile_pool(name="onat", bufs=2))    # [128,768] f32
        lnsp = ctx.enter_context(tc.tile_pool(name="lns", bufs=3))
        dramp = ctx.enter_context(tc.tile_pool(name="dram", bufs=1, space="DRAM"))
        ps_s = ctx.enter_context(tc.tile_pool(name="ps_s", bufs=2, space="PSUM"))
        ps_sum = ctx.enter_context(tc.tile_pool(name="ps_sum", bufs=1, space="PSUM"))
        ps_acc = ctx.enter_context(tc.tile_pool(name="ps_acc", bufs=4, space="PSUM"))
        if True:
            # ---------- constants / params ----------
            ident = constp.tile([128, 128], BF16)
            make_identity(nc, ident)
            ones = constp.tile([128, 1], BF16)
            nc.vector.memset(ones[:], 1.0)
            epsbn = constp.tile([128, 1], F32)
            nc.vector.memset(epsbn[:], EPS_BN)
            epsln = constp.tile([128, 1], F32)
            nc.vector.memset(epsln[:], EPS_LN)

            def load_pt(t):  # [D] -> [128, DT] with col et, row p = t[et*128+p]
                s = constp.tile([128, DT], F32)
                nc.scalar.dma_start(out=s[:], in_=t.ap().rearrange("(t p) -> p t", p=128))
                return s

            ba_s, bv_s = load_pt(ba_d), load_pt(bv_d)
            bnag_s, bnab_s = load_pt(bnag_d), load_pt(bnab_d)
            bnvg_s, bnvb_s = load_pt(bnvg_d), load_pt(bnvb_d)
            pa_s = constp.tile([128, 1], F32)
            nc.gpsimd.dma_start(out=pa_s[:], in_=bcast_ap(pa_d, 1))
            pv_s = constp.tile([128, 1], F32)
            nc.gpsimd.dma_start(out=pv_s[:], in_=bcast_ap(pv_d, 1))
            lng_s = constp.tile([128, D], F32)
            nc.gpsimd.dma_start(out=lng_s[:], in_=bcast_ap(lng_d, D))
            lnb_s = constp.tile([128, D], F32)
            nc.gpsimd.dma_start(out=lnb_s[:], in_=bcast_ap(lnb_d, D))

            # DRAM bounce buffers
            yD0 = dramp.tile([128, DT, L], BF16, tag="yD0")
            yD1 = dramp.tile([128, DT, L], BF16, tag="yD1")
            arin0 = dramp.tile([128, 2 * DT], F32, tag="arin0")
            arin1 = dramp.tile([128, 2 * DT], F32, tag="arin1")
            arout0 = dramp.tile([128, 2 * DT], F32, tag="arout0")
            arout1 = dramp.tile([128, 2 * DT], F32, tag="arout1")
            yD = {0: yD0, 1: yD1}
            arin = {0: arin0, 1: arin1}
            arout = {0: arout0, 1: arout1}

            # ---------- input cast-loads (gpsimd SWDGE ring) ----------
            # Chunked: 4 l-tiles (512 rows) per descriptor.
            def cast_chunk(pool, src_d, c, nt, name):
                t = pool.tile([128, nt, D], BF16, tag=pool.name, name=name)
                src = src_d.ap()[c * nt * 128:(c + 1) * nt * 128, :]
                nc.gpsimd.dma_start(
                    out=t[:], in_=src.rearrange("(t p) d -> p t d", p=128))
                return t

            natq = [None] * NCH
            nata = [None] * NCH
            natv = [None] * NCH
            wnat = {}
            natq[0] = cast_chunk(stgp, xq_d, 0, CH, "natq0")
            nata[0] = cast_chunk(natap, xa_d, 0, CH, "nata0")
            nata[1] = cast_chunk(natap, xa_d, 1, CH, "nata1")
            wnat[(0, 0)] = cast_chunk(stgp, Wa_d, 0, WCH, "wa0")
            wnat[(0, 1)] = cast_chunk(stgp, Wa_d, 1, WCH, "wa1")
            natq[1] = cast_chunk(stgp, xq_d, 1, CH, "natq1")
            nata[2] = cast_chunk(natap, xa_d, 2, CH, "nata2")
            nata[3] = cast_chunk(natap, xa_d, 3, CH, "nata3")
            natq[2] = cast_chunk(stgp, xq_d, 2, CH, "natq2")
            natq[3] = cast_chunk(stgp, xq_d, 3, CH, "natq3")
            for c in range(NCH):
                natv[c] = cast_chunk(natvp, xv_d, c, CH, f"natv{c}")
            wnat[(1, 0)] = cast_chunk(stgp, Wv_d, 0, WCH, "wv0")
            wnat[(1, 1)] = cast_chunk(stgp, Wv_d, 1, WCH, "wv1")

            def nat_slice(chunks, g):          # [128, 768] natural l-tile g
                return chunks[g // CH][:, g % CH, :]

            # ---------- transposes (ALL on the sync HWDGE ring) ----------
            # The xbar is shared hardware: concurrent transposes from two
            # HWDGE rings corrupt sub-blocks. Keep every transpose on one
            # ring, ordered to match the consumption order of branch_a.
            kvT_a = [kvtp.tile([128, DT, 128], BF16, tag="kvT", name=f"kvTa{c}")
                     for c in range(MT)]
            kvT_v = [kvtp.tile([128, DT, 128], BF16, tag="kvT", name=f"kvTv{c}")
                     for c in range(MT)]
            qT = [qaccp.tile([128, DT, LBS], BF16, tag="qacc", name=f"qT{i}")
                  for i in range(NLB)]

            def tq(c):
                nc.sync.dma_start_transpose(
                    qT[c // CH][:, :, (c % CH) * 128:(c % CH + 1) * 128],
                    nat_slice(natq, c))

            def wt_transpose(bi):
                WT = wtp.tile([128, DT, D], BF16, tag="WT", name=f"WT{bi}")
                for c6 in range(DT):
                    nc.sync.dma_start_transpose(
                        WT[:, :, c6 * 128:(c6 + 1) * 128],
                        wnat[(bi, c6 // WCH)][:, c6 % WCH, :])
                return WT

            for c in range(CH):
                tq(c)
            for c in range(8):
                nc.sync.dma_start_transpose(kvT_a[c][:, :, :], nat_slice(nata, c))
            WTa = wt_transpose(0)
            for c in range(8, MT):
                nc.sync.dma_start_transpose(kvT_a[c][:, :, :], nat_slice(nata, c))
            for c in range(CH, LT):
                tq(c)
            for c in range(MT):
                nc.sync.dma_start_transpose(kvT_v[c][:, :, :], nat_slice(natv, c))
            WTv = wt_transpose(1)

            # ---------- branch compute (S -> softmax -> ctx -> y -> stats) ----
            def branch_compute(bi, WT, b_s, kvT, nat_kv):
                statsr = statp.tile([128, DT, NLB, 6], F32, tag=f"statsr{bi}")
                for lb in range(NLB):
                    E = ep.tile([128, MT, LBS], BF16, tag="E")
                    for mt in range(MT):
                        S = ps_s.tile([128, LBS], F32, tag="S")
                        for dt in range(DT):
                            nc.tensor.matmul(
                                S[:], kvT[mt][:, dt, :], qT[lb][:, dt, :],
                                start=(dt == 0), stop=(dt == DT - 1))
                        nc.scalar.activation(out=E[:, mt, :], in_=S[:],
                                             func=AF.Exp, scale=SCALE)
                    s_ps = ps_sum.tile([1, LBS], F32, tag="ssum")
                    for mt in range(MT):
                        nc.tensor.matmul(s_ps[:], ones[:], E[:, mt, :],
                                         start=(mt == 0), stop=(mt == MT - 1))
                    rsb = smallp.tile([1, LBS], F32, tag="rsb")
                    nc.vector.reciprocal(rsb[:], s_ps[:])
                    rbd = dramp.tile([LBS], F32, tag="rbd")
                    nc.scalar.dma_start(out=rbd[:], in_=rsb[:])
                    rbc = rbp.tile([128, LBS], F32, tag="rbc")
                    nc.scalar.dma_start(out=rbc[:], in_=bcast_ap(rbd, LBS))

                    ctx_sb = []
                    for dt in range(DT):
                        cps = ps_acc.tile([128, LBS], F32, tag="acc")
                        for mt in range(MT):
                            nc.tensor.matmul(
                                cps[:], nat_slice(nat_kv, mt)[:, dt * 128:(dt + 1) * 128],
                                E[:, mt, :], start=(mt == 0), stop=(mt == MT - 1))
                        csb = ctxp.tile([128, LBS], BF16, tag="ctx")
                        nc.vector.tensor_copy(csb[:], cps[:])
                        ctx_sb.append(csb)
                    ysb = ysp.tile([128, DT, LBS], BF16, tag="ys")
                    for et in range(DT):
                        yps = ps_acc.tile([128, LBS], F32, tag="acc")
                        for dt in range(DT):
                            nc.tensor.matmul(
                                yps[:], WT[:, dt, et * 128:(et + 1) * 128],
                                ctx_sb[dt][:], start=(dt == 0), stop=(dt == DT - 1))
                        nc.vector.tensor_mul(ysb[:, et, :], yps[:], rbc[:])
                        nc.vector.tensor_scalar(
                            out=ysb[:, et, :], in0=ysb[:, et, :],
                            scalar1=b_s[:, et:et + 1], scalar2=None, op0=ALU.add)
                        nc.vector.bn_stats(out=statsr[:, et, lb, :], in_=ysb[:, et, :])
                    nc.gpsimd.dma_start(
                        out=yD[bi][:, :, lb * LBS:(lb + 1) * LBS], in_=ysb[:])

                # per-core stats -> sums -> AllReduce kickoff
                mv = smallp.tile([128, DT, 2], F32, tag=f"mv{bi}")
                for et in range(DT):
                    nc.vector.bn_aggr(out=mv[:, et, :], in_=statsr[:, et, :, :])
                arin_s = smallp.tile([128, 2 * DT], F32, tag=f"ari{bi}")
                nc.vector.tensor_scalar(
                    out=arin_s[:, 0:DT], in0=mv[:, :, 0], scalar1=float(L),
                    scalar2=None, op0=ALU.mult)
                tmp = smallp.tile([128, DT], F32, tag=f"tmp{bi}")
                nc.vector.tensor_mul(tmp[:], mv[:, :, 0], mv[:, :, 0])
                nc.vector.tensor_add(tmp[:], tmp[:], mv[:, :, 1])
                nc.vector.tensor_scalar(
                    out=arin_s[:, DT:2 * DT], in0=tmp[:], scalar1=float(L),
                    scalar2=None, op0=ALU.mult)
                nc.scalar.dma_start(out=arin[bi][:], in_=arin_s[:])
                nc.gpsimd.collective_compute(
                    "AllReduce", ALU.add,
                    replica_groups=[list(range(N_CORES))],
                    ins=[arin[bi].opt()], outs=[arout[bi].opt()])

            # ---------- BN coefficients from the AllReduced stats ----------
            def branch_coefs(bi, bng_s, bnb_s, alpha_s):
                gs = smallp.tile([128, 2 * DT], F32, tag=f"gs{bi}")
                nc.scalar.dma_start(out=gs[:], in_=arout[bi][:])
                inv_n = 1.0 / float(L * N_CORES)
                gm = smallp.tile([128, DT], F32, tag=f"gm{bi}")
                nc.vector.tensor_scalar(out=gm[:], in0=gs[:, 0:DT],
                                        scalar1=inv_n, scalar2=None, op0=ALU.mult)
                gvar = smallp.tile([128, DT], F32, tag=f"gv{bi}")
                nc.vector.tensor_scalar(out=gvar[:], in0=gs[:, DT:2 * DT],
                                        scalar1=inv_n, scalar2=None, op0=ALU.mult)
                tmp2 = smallp.tile([128, DT], F32, tag=f"t2{bi}")
                nc.vector.tensor_mul(tmp2[:], gm[:], gm[:])
                nc.vector.tensor_sub(gvar[:], gvar[:], tmp2[:])
                std = smallp.tile([128, DT], F32, tag=f"sd{bi}")
                nc.scalar.activation(out=std[:], in_=gvar[:], func=AF.Sqrt,
                                     bias=epsbn[:], scale=1.0)
                rstd = smallp.tile([128, DT], F32, tag=f"rs{bi}")
                nc.vector.reciprocal(rstd[:], std[:])
                sc1 = smallp.tile([128, DT], F32, tag=f"s1{bi}")
                nc.vector.tensor_mul(sc1[:], bng_s[:], rstd[:])
                sh1 = smallp.tile([128, DT], F32, tag=f"h1{bi}")
                nc.vector.tensor_mul(sh1[:], gm[:], sc1[:])
                nc.vector.tensor_sub(sh1[:], bnb_s[:], sh1[:])
                sc2 = smallp.tile([128, DT], F32, tag=f"s2{bi}")
                nc.vector.tensor_scalar(out=sc2[:], in0=sc1[:], scalar1=alpha_s[:],
                                        scalar2=-1.0, op0=ALU.mult, op1=ALU.mult)
                sh2 = smallp.tile([128, DT], F32, tag=f"h2{bi}")
                nc.vector.tensor_scalar(out=sh2[:], in0=sh1[:], scalar1=alpha_s[:],
                                        scalar2=-1.0, op0=ALU.mult, op1=ALU.mult)
                return sc1, sh1, sc2, sh2

            # ---------- BN + PReLU apply from the DRAM bounce ----------
            def branch_apply(bi, coefs, acc_tiles):
                sc1, sh1, sc2, sh2 = coefs
                if bi == 0:
                    for et in range(DT):
                        acc_tiles[et] = qaccp.tile([128, L], BF16, tag="qacc",
                                                   name=f"accsb{et}")
                for lc in range(NLB):
                    lsl = slice(lc * LBS, (lc + 1) * LBS)
                    ybk = ybkp.tile([128, DT, LBS], BF16, tag="ybk")
                    nc.gpsimd.dma_start(out=ybk[:], in_=yD[bi][:, :, lsl])
                    for et in range(DT):
                        r1 = rtp.tile([128, LBS], BF16, tag="rt")
                        nc.scalar.activation(out=r1[:], in_=ybk[:, et, :], func=AF.Relu,
                                             scale=sc1[:, et:et + 1], bias=sh1[:, et:et + 1])
                        r2 = rtp.tile([128, LBS], BF16, tag="rt")
                        nc.scalar.activation(out=r2[:], in_=ybk[:, et, :], func=AF.Relu,
                                             scale=sc2[:, et:et + 1], bias=sh2[:, et:et + 1])
                        if bi == 0:
                            nc.vector.tensor_sub(acc_tiles[et][:, lsl], r1[:], r2[:])
                        else:
                            nc.vector.tensor_add(acc_tiles[et][:, lsl],
                                                 acc_tiles[et][:, lsl], r1[:])
                            nc.vector.tensor_sub(acc_tiles[et][:, lsl],
                                                 acc_tiles[et][:, lsl], r2[:])

            branch_compute(0, WTa, ba_s, kvT_a, nata)
            branch_compute(1, WTv, bv_s, kvT_v, natv)
            coefs_a = branch_coefs(0, bnag_s, bnab_s, pa_s)
            acc_tiles = [None] * DT
            branch_apply(0, coefs_a, acc_tiles)
            coefs_v = branch_coefs(1, bnvg_s, bnvb_s, pv_s)
            branch_apply(1, coefs_v, acc_tiles)

            # ---------- tail: transpose + residual + LayerNorm ----------
            xr = [None] * (LT // 2)
            for lt in range(LT):
                if lt % 2 == 0:
                    xr[lt // 2] = xrp.tile([128, 2, D], F32, tag="xr",
                                           name=f"xr{lt // 2}")
                    src = xq_d.ap()[lt * 128:(lt + 2) * 128, :]
                    nc.gpsimd.dma_start(
                        out=xr[lt // 2][:], in_=src.rearrange("(t p) d -> p t d", p=128))
                xrt = xr[lt // 2][:, lt % 2, :]
                onat = onatp.tile([128, D], F32, tag="onat")
                for dt in range(DT):
                    tp = ps_s.tile([128, 128], BF16, tag="S")
                    nc.tensor.transpose(tp[:], acc_tiles[dt][:, lt * 128:(lt + 1) * 128],
                                        ident[:])
                    nc.vector.tensor_add(onat[:, dt * 128:(dt + 1) * 128], tp[:],
                                         xrt[:, dt * 128:(dt + 1) * 128])
                lns = lnsp.tile([128, 3, 6], F32, tag="lns")
                for g3 in range(3):
                    nc.vector.bn_stats(out=lns[:, g3, :],
                                       in_=onat[:, g3 * 256:(g3 + 1) * 256])
                mvl = lnsp.tile([128, 2], F32, tag="mvl")
                nc.vector.bn_aggr(out=mvl[:], in_=lns[:])
                stdl = lnsp.tile([128, 1], F32, tag="stdl")
                nc.scalar.activation(out=stdl[:], in_=mvl[:, 1:2], func=AF.Sqrt,
                                     bias=epsln[:], scale=1.0)
                rstdl = lnsp.tile([128, 1], F32, tag="rstdl")
                nc.vector.reciprocal(rstdl[:], stdl[:])
                nc.vector.tensor_scalar(out=onat[:], in0=onat[:],
                                        scalar1=mvl[:, 0:1], scalar2=rstdl[:],
                                        op0=ALU.subtract, op1=ALU.mult)
                nc.vector.tensor_mul(onat[:], onat[:], lng_s[:])
                nc.vector.tensor_add(onat[:], onat[:], lnb_s[:])
                nc.sync.dma_start(out=out_d.ap()[lt * 128:(lt + 1) * 128, :], in_=onat[:])

    nc.compile()
    return nc


def _get_nc():
    global _CACHED_NC
    if _CACHED_NC is None:
        _CACHED_NC = _build_nc()
    return _CACHED_NC


def kernel(**inputs):
    nc = _get_nc()
    x_a = np.asarray(inputs["x_a"], np.float32)
    x_v = np.asarray(inputs["x_v"], np.float32)
    x = np.asarray(inputs["x"], np.float32)
    shared = {
        "Wa": np.ascontiguousarray(inputs["W_a"], np.float32),
        "Wv": np.ascontiguousarray(inputs["W_v"], np.float32),
        "ba": np.ascontiguousarray(inputs["b_a"], np.float32),
        "bv": np.ascontiguousarray(inputs["b_v"], np.float32),
        "bnag": np.ascontiguousarray(inputs["bn_a_g"], np.float32),
        "bnab": np.ascontiguousarray(inputs["bn_a_b"], np.float32),
        "bnvg": np.ascontiguousarray(inputs["bn_v_g"], np.float32),
        "bnvb": np.ascontiguousarray(inputs["bn_v_b"], np.float32),
        "pa": np.ascontiguousarray(inputs["prelu_a"], np.float32),
        "pv": np.ascontiguousarray(inputs["prelu_v"], np.float32),
        "lng": np.ascontiguousarray(inputs["ln_g"], np.float32),
        "lnb": np.ascontiguousarray(inputs["ln_b"], np.float32),
    }
    in_maps = []
    for b in range(N_CORES):
        m = dict(shared)
        m["xq"] = np.ascontiguousarray(x[:, b, :])
        m["xa"] = np.ascontiguousarray(x_a[:, b, :])
        m["xv"] = np.ascontiguousarray(x_v[:, b, :])
        in_maps.append(m)
    trace = bool(int(os.environ.get("COATT_TRACE", "0")))
    res = run_bass_kernel_spmd(nc, in_maps, core_ids=list(range(N_CORES)),
                               trace=trace)
    kernel.last_results = res
    out = np.stack([res.results[b]["out"] for b in range(N_CORES)], axis=1)
    return out.astype(np.float32)
